# revision 1
# baseline (speedup 1.0000x reference)
"""Binned spectra (per-row histogram) Trainium2 kernel.

Algorithm (per batch row, 9900 = 100*99 bins):
  bin = trunc((mz-10)/0.1)  in [0, 9900) for valid peaks
  hi = bin // 99  in [0, 100);  lo = bin - 99*hi  in [0, 99)
  hist[hi, lo] = sum_p val_p * d(hi_p==hi) * d(lo_p==lo)   (outer product, PE matmul)
  where val_p = sqrt(intensity_p) * (10 <= mz_p < 1000)

The PE computes hist = A^T @ C with A[p,hi] = d(hi_p==hi) (bf16 one-hot,
stationary) and C[p,lo] = d(lo_p==lo)*val_p.  For fp32 accuracy val is split
val = bf16(val) + bf16(val - bf16(val)); both bf16 pieces are streamed as
separate moving operands accumulating into the same fp32 PSUM tile, so the
result carries ~16 mantissa bits per value (measured rel err ~2e-6 vs the
fp32 reference).  The fp32 division (mz-10)/0.1f is emulated exactly
(Dekker residual + 1-eps correction) because the DVE has no divide ALU op,
and floor() is computed robustly to the hardware's f32->i32 rounding mode.

One-hot mask builds dominate; they are round-robined across DVE (~54%),
Pool/gpsimd (~28%) and ACT (~25% of the hi-masks, via |x| then relu(1-x)).

Data parallel over 8 NeuronCores: each core takes 512 of the 4096 rows.
"""

import sys

sys.path.insert(0, "/opt/trn_rl_repo")

import numpy as np

import concourse.bass as bass
import concourse.tile as tile
from concourse import bacc, mybir
from concourse.bass_utils import run_bass_kernel_spmd
from concourse.masks import make_identity

N_CORES = 8
B, P = 4096, 1024
NUM_BINS = 9900
H, L = 100, 99
RT = 128  # rows per row-tile (SBUF partition dim)
NCHUNK = P // 128  # peak chunks per row

f32 = mybir.dt.float32
bf16 = mybir.dt.bfloat16
i32 = mybir.dt.int32

INV99 = float(np.float32(1.0) / np.float32(99.0))
# u / 0.1f == u * 10 * (1 - EPS_D) exactly:  10*0.1f = 1 + 1.49e-8
EPS_D = float(1.0 - 1.0 / (10.0 * np.float64(np.float32(0.1))))


def build_program(
    rows_per_core: int,
    exact: bool = True,
    # Real-HW measured (chained-execution bench): gpsimd tensor_scalar is
    # ~10x the cost model, so Pool gets NO mask work; DVE ~60% / ACT ~40%.
    pool_pat: tuple = (1, ()),
    mask_bufs: int = 24,
    mm_bufs: int = 3,
    act_pat: tuple = (5, (0, 2)),
    act_c1: bool = False,
    stage_pat: tuple = (1, (0,)),
    stage_rows: int = 2,
    fuse_c: bool = False,
    scr_bufs: int = 1,
):
    """Build the (single-core SPMD) Bass program for rows_per_core rows.

    exact=True: values enter the bf16 matmul split in two bf16 pieces
    (fp32-accurate result, ~rel 3e-6).  exact=False: single bf16 piece
    (~rel 5e-4) but ~30% fewer mask ops.
    """
    from contextlib import ExitStack

    assert rows_per_core % RT == 0
    nt = rows_per_core // RT
    cw = 2 * H if exact else H  # moving-operand width per chunk

    nc = bacc.Bacc(
        "TRN2", target_bir_lowering=False, debug=False, num_devices=N_CORES
    )
    mz_d = nc.dram_tensor("mz", [rows_per_core, P], f32, kind="ExternalInput").ap()
    it_d = nc.dram_tensor(
        "intensities", [rows_per_core, P], f32, kind="ExternalInput"
    ).ap()
    out_d = nc.dram_tensor(
        "out", [rows_per_core, NUM_BINS], f32, kind="ExternalOutput"
    ).ap()

    with tile.TileContext(nc) as tc, ExitStack() as ctx:
        cpool = ctx.enter_context(tc.tile_pool(name="consts", bufs=1))
        inpool = ctx.enter_context(tc.tile_pool(name="inp", bufs=2))
        tpsum = ctx.enter_context(tc.tile_pool(name="tpsum", bufs=2, space="PSUM"))
        scr = ctx.enter_context(tc.tile_pool(name="scratch", bufs=scr_bufs))
        wide = ctx.enter_context(tc.tile_pool(name="wide", bufs=2))
        maskp = ctx.enter_context(tc.tile_pool(name="masks", bufs=mask_bufs))
        mmpsum = ctx.enter_context(
            tc.tile_pool(name="mmpsum", bufs=mm_bufs, space="PSUM")
        )
        histp = ctx.enter_context(tc.tile_pool(name="hist", bufs=1))

        # constants
        ident = cpool.tile([128, 128], f32, tag="ident")
        make_identity(nc, ident[:])
        iota_i = cpool.tile([128, H], i32, tag="iota_i")
        nc.gpsimd.iota(iota_i[:], pattern=[[1, H]], base=0, channel_multiplier=0)
        iota_bf = cpool.tile([128, H], bf16, tag="iota_bf")
        nc.vector.tensor_copy(iota_bf[:], iota_i[:])
        if fuse_c:
            # (0,0,1,1,...,99,99) for the interleaved fused C build
            iota2_i = cpool.tile([128, 2 * H], i32, tag="iota2_i")
            nc.gpsimd.iota(
                iota2_i[:], pattern=[[1, H], [0, 2]], base=0, channel_multiplier=0
            )
            iota2_bf = cpool.tile([128, 2 * H], bf16, tag="iota2_bf")
            nc.vector.tensor_copy(iota2_bf[:], iota2_i[:])

        out_v = out_d.rearrange("(t r) (h l) -> t h r l", r=RT, l=L)

        for t in range(nt):
            rs = t * RT
            mzt = inpool.tile([128, P], f32, tag="mz")
            nc.sync.dma_start(mzt[:], mz_d[rs : rs + RT, :])
            itt = inpool.tile([128, P], f32, tag="it")
            nc.sync.dma_start(itt[:], it_d[rs : rs + RT, :])

            # ---- transpose to peak-major + per-chunk math ----
            uT = scr.tile([128, P], f32, tag="uT")  # mz-10, peak-major
            sqT = scr.tile([128, P], f32, tag="sqT")  # sqrt(intensity)
            for c in range(NCHUNK):
                cs = slice(c * 128, (c + 1) * 128)
                pz = tpsum.tile([128, 128], f32, tag="tp")
                nc.tensor.transpose(pz[:], mzt[:, cs], ident[:])
                nc.vector.tensor_scalar(
                    uT[:, cs], pz[:], 10.0, None, mybir.AluOpType.subtract
                )
                pz2 = tpsum.tile([128, 128], f32, tag="tp")
                nc.tensor.transpose(pz2[:], itt[:, cs], ident[:])
                nc.scalar.sqrt(sqT[:, cs], pz2[:])

            # ---- wide math (all peak-major [128, 1024]) ----
            # q = u / 0.1 (exactly-rounded fp32 division, matches reference).
            # The DVE has no divide ALU op; emulate:  u/0.1f = u*10*(1-eps)
            # with q_hi = RN(8u+2u) (8u, 2u exact), Dekker residual
            # e = 2u - (q_hi - 8u), correction c2 = e - eps*q_hi, and
            # q = RN(q_hi + c2) == RN(u/0.1f).
            b2 = scr.tile([128, P], f32, tag="b2")
            nc.vector.tensor_scalar(b2[:], uT[:], 2.0, None, mybir.AluOpType.mult)
            qhi = scr.tile([128, P], f32, tag="qhi")
            nc.vector.scalar_tensor_tensor(
                qhi[:],
                in0=uT[:],
                scalar=8.0,
                in1=b2[:],
                op0=mybir.AluOpType.mult,
                op1=mybir.AluOpType.add,
            )
            bv = scr.tile([128, P], f32, tag="bv")
            nc.vector.scalar_tensor_tensor(
                bv[:],
                in0=uT[:],
                scalar=-8.0,
                in1=qhi[:],
                op0=mybir.AluOpType.mult,
                op1=mybir.AluOpType.add,
            )
            ederr = scr.tile([128, P], f32, tag="a8")
            nc.vector.tensor_tensor(ederr[:], b2[:], bv[:], mybir.AluOpType.subtract)
            c2t = scr.tile([128, P], f32, tag="bv")
            nc.vector.scalar_tensor_tensor(
                c2t[:],
                in0=qhi[:],
                scalar=-EPS_D,
                in1=ederr[:],
                op0=mybir.AluOpType.mult,
                op1=mybir.AluOpType.add,
            )
            qT = scr.tile([128, P], f32, tag="qT")
            nc.vector.tensor_tensor(qT[:], qhi[:], c2t[:], mybir.AluOpType.add)
            # binf = floor(q) robust to convert rounding mode
            itmp = scr.tile([128, P], i32, tag="itmp")
            nc.vector.tensor_copy(itmp[:], qT[:])
            ftmp = scr.tile([128, P], f32, tag="ftmp")
            nc.scalar.copy(ftmp[:], itmp[:])
            cond = scr.tile([128, P], f32, tag="cond")
            nc.vector.tensor_tensor(cond[:], ftmp[:], qT[:], mybir.AluOpType.is_gt)
            binT = scr.tile([128, P], f32, tag="binT")
            nc.vector.tensor_tensor(binT[:], ftmp[:], cond[:], mybir.AluOpType.subtract)
            # hi = floor((bin + 0.5) / 99)
            hq = scr.tile([128, P], f32, tag="qT")
            nc.vector.tensor_scalar(
                hq[:], binT[:], 0.5, INV99, mybir.AluOpType.add, mybir.AluOpType.mult
            )
            itmp2 = scr.tile([128, P], i32, tag="itmp")
            nc.vector.tensor_copy(itmp2[:], hq[:])
            ftmp2 = scr.tile([128, P], f32, tag="ftmp")
            nc.scalar.copy(ftmp2[:], itmp2[:])
            cond2 = scr.tile([128, P], f32, tag="cond")
            nc.vector.tensor_tensor(cond2[:], ftmp2[:], hq[:], mybir.AluOpType.is_gt)
            hiT = wide.tile([128, P], f32, tag="hiT")
            nc.vector.tensor_tensor(
                hiT[:], ftmp2[:], cond2[:], mybir.AluOpType.subtract
            )
            # lo = bin - 99*hi
            loT = wide.tile([128, P], f32, tag="loT")
            nc.vector.scalar_tensor_tensor(
                loT[:],
                in0=hiT[:],
                scalar=-99.0,
                in1=binT[:],
                op0=mybir.AluOpType.mult,
                op1=mybir.AluOpType.add,
            )
            # val = sqrt(it) * (u >= 0) * (u < 990)
            v1 = scr.tile([128, P], f32, tag="v1")
            nc.vector.scalar_tensor_tensor(
                v1[:],
                in0=uT[:],
                scalar=0.0,
                in1=sqT[:],
                op0=mybir.AluOpType.is_ge,
                op1=mybir.AluOpType.mult,
            )
            vT = scr.tile([128, P], f32, tag="vT")
            nc.vector.scalar_tensor_tensor(
                vT[:],
                in0=uT[:],
                scalar=990.0,
                in1=v1[:],
                op0=mybir.AluOpType.is_lt,
                op1=mybir.AluOpType.mult,
            )
            # val splitting for fp32 accuracy: vhif = f32(bf16(v)) is exactly
            # bf16-representable, so mask ops' bf16 output conversion is exact
            # on any engine; vlo = v - vhif carries the residual.
            vhib = scr.tile([128, P], bf16, tag="vhib")
            nc.scalar.copy(vhib[:], vT[:])
            vhif = wide.tile([128, P], f32, tag="vhif")
            nc.scalar.copy(vhif[:], vhib[:])
            vloT = wide.tile([128, P], f32, tag="vloT")
            nc.vector.tensor_tensor(
                vloT[:], vT[:], vhif[:], mybir.AluOpType.subtract
            )
            if act_c1:
                nvhi = wide.tile([128, P], f32, tag="nvhi")
                nc.vector.tensor_scalar(
                    nvhi[:], vhif[:], -1.0, None, mybir.AluOpType.mult
                )
            if fuse_c:
                # interleaved (vhi, vlo) pairs per peak, bf16
                v2 = wide.tile([128, 2 * P], bf16, tag="v2")
                v2v = v2[:].rearrange("p (c two) -> p c two", two=2)
                nc.scalar.copy(v2v[:, :, 0], vhif[:])
                nc.scalar.copy(v2v[:, :, 1], vloT[:])

            # ---- per-row one-hot matmuls ----
            # Mask builds are the bottleneck: split them DVE / Pool (gpsimd).
            # Both val-pieces accumulate into the same PSUM tile (hardware
            # allows only one PSUM operand per vector op, so no psum+psum add).
            hist = histp.tile([100, RT * H], f32, tag="hist")
            pr_full = None
            for r in range(RT):
                # stage_rows consecutive rows share one PSUM tile (one bank
                # per row, 512-f32 pitch) so staging copies can batch.
                if r % stage_rows == 0:
                    pr_full = mmpsum.tile([100, stage_rows * 512], f32, tag="mm")
                roff = (r % stage_rows) * 512
                pr = pr_full[:, roff : roff + H]
                for c in range(NCHUNK):
                    col = c * 128 + r
                    idx = r * NCHUNK + c
                    # round-robin engines: Pool takes a fraction of mask ops
                    eng = (
                        nc.gpsimd
                        if idx % pool_pat[0] in pool_pat[1]
                        else nc.vector
                    )
                    a = maskp.tile([128, H], bf16, tag="A")
                    if act_pat and idx % act_pat[0] in act_pat[1]:
                        # build the hi one-hot on the ACT engine:
                        # tmp = |hi - iota|; a = relu(1 - tmp)
                        at = maskp.tile([128, H], bf16, tag="At")
                        nc.scalar.activation(
                            at[:],
                            iota_bf[:],
                            mybir.ActivationFunctionType.Abs,
                            bias=hiT[:, col : col + 1],
                            scale=-1.0,
                        )
                        nc.scalar.activation(
                            a[:],
                            at[:],
                            mybir.ActivationFunctionType.Relu,
                            bias=1.0,
                            scale=-1.0,
                        )
                    else:
                        eng.tensor_scalar(
                            a[:],
                            iota_bf[:],
                            hiT[:, col : col + 1],
                            None,
                            mybir.AluOpType.is_equal,
                        )
                    if fuse_c and exact:
                        # one fused op builds delta(lo)*{vhi, vlo} interleaved
                        c12i = maskp.tile([128, 2 * H], bf16, tag="C")
                        civ = c12i[:].rearrange("p (j two) -> p j two", two=2)
                        eng.scalar_tensor_tensor(
                            civ,
                            in0=iota2_bf[:].rearrange(
                                "p (j two) -> p j two", two=2
                            ),
                            scalar=loT[:, col : col + 1],
                            in1=v2[:]
                            .rearrange("p (c two) -> p c two", two=2)[
                                :, col : col + 1, :
                            ]
                            .to_broadcast([128, H, 2]),
                            op0=mybir.AluOpType.is_equal,
                            op1=mybir.AluOpType.mult,
                        )
                        nc.tensor.matmul(
                            pr[:, :],
                            lhsT=a[:],
                            rhs=civ[:, :, 0],
                            start=(c == 0),
                            stop=False,
                        )
                        nc.tensor.matmul(
                            pr[:, :],
                            lhsT=a[:],
                            rhs=civ[:, :, 1],
                            start=False,
                            stop=(c == NCHUNK - 1),
                        )
                        continue
                    c12 = maskp.tile([128, cw], bf16, tag="C")
                    if act_c1 and act_pat and idx % act_pat[0] in act_pat[1]:
                        # C1 on ACT: t = |lo - iota|; c1 = relu(vhi - vhi*t)
                        at2 = maskp.tile([128, H], bf16, tag="At2")
                        nc.scalar.activation(
                            at2[:],
                            iota_bf[:],
                            mybir.ActivationFunctionType.Abs,
                            bias=loT[:, col : col + 1],
                            scale=-1.0,
                        )
                        nc.scalar.activation(
                            c12[:, 0:H],
                            at2[:],
                            mybir.ActivationFunctionType.Relu,
                            bias=vhif[:, col : col + 1],
                            scale=nvhi[:, col : col + 1],
                        )
                    else:
                        eng.tensor_scalar(
                            c12[:, 0:H],
                            iota_bf[:],
                            loT[:, col : col + 1],
                            vhif[:, col : col + 1],
                            mybir.AluOpType.is_equal,
                            mybir.AluOpType.mult,
                        )
                    if exact:
                        eng.tensor_scalar(
                            c12[:, H : 2 * H],
                            iota_bf[:],
                            loT[:, col : col + 1],
                            vloT[:, col : col + 1],
                            mybir.AluOpType.is_equal,
                            mybir.AluOpType.mult,
                        )
                    nc.tensor.matmul(
                        pr[:, :],
                        lhsT=a[:],
                        rhs=c12[:, 0:H],
                        start=(c == 0),
                        stop=(c == NCHUNK - 1) and not exact,
                    )
                    if exact:
                        nc.tensor.matmul(
                            pr[:, :],
                            lhsT=a[:],
                            rhs=c12[:, H : 2 * H],
                            start=False,
                            stop=(c == NCHUNK - 1),
                        )
                # PSUM -> SBUF staging (batched over stage_rows), ACT/DVE split
                if (r + 1) % stage_rows == 0:
                    r0 = r + 1 - stage_rows
                    src = pr_full[:].rearrange("p (b x) -> p b x", x=512)[
                        :, :, 0:H
                    ]
                    dst = hist[:, r0 * H : (r + 1) * H]
                    if (r // stage_rows) % stage_pat[0] in stage_pat[1]:
                        nc.scalar.copy(dst, src)
                    else:
                        nc.vector.tensor_copy(dst, src)

            histv = hist[:].rearrange("h (r l) -> h r l", l=H)[:, :, 0:L]
            nc.sync.dma_start(out_v[t], histv)

    nc.compile()
    return nc


_CACHE: dict[int, object] = {}


def _get_program(rows_per_core: int):
    if rows_per_core not in _CACHE:
        _CACHE[rows_per_core] = build_program(rows_per_core)
    return _CACHE[rows_per_core]


def kernel(mz: np.ndarray, intensities: np.ndarray, trace: bool = False):
    mz = np.ascontiguousarray(np.asarray(mz, dtype=np.float32))
    intensities = np.ascontiguousarray(np.asarray(intensities, dtype=np.float32))
    bb = mz.shape[0]
    rows_per_core = bb // N_CORES
    nc = _get_program(rows_per_core)
    in_maps = []
    for i in range(N_CORES):
        sl = slice(i * rows_per_core, (i + 1) * rows_per_core)
        in_maps.append({"mz": mz[sl], "intensities": intensities[sl]})
    try:
        res = run_bass_kernel_spmd(
            nc, in_maps, core_ids=list(range(N_CORES)), trace=trace
        )
    except ModuleNotFoundError:
        # NTFF profiling hook unavailable in this environment
        res = run_bass_kernel_spmd(
            nc, in_maps, core_ids=list(range(N_CORES)), trace=False
        )
    out = np.concatenate([res.results[i]["out"] for i in range(N_CORES)], axis=0)
    if trace:
        kernel.last_exec_time_ns = res.exec_time_ns
        kernel.last_results = res
    return out


kernel.last_exec_time_ns = None



# revision 2
# speedup vs baseline: 2.6464x; 2.6464x over previous
"""Binned spectra (per-row histogram) Trainium2 kernel — v2.

Algorithm per batch row (9900 = 100*99 bins):
  bin = trunc((mz-10)/0.1) in [0, 9900) for valid peaks
  hi = bin // 99, lo = bin - 99*hi
  hist[hi, lo] = sum_p v_p * d(hi_p==hi) * d(lo_p==lo)   (PE matmul outer product)
  v_p = sqrt(intensity_p) * (10 <= mz_p < 1000)

v2 cost-model-driven design:
  - single fp16 val piece (rel err ~5e-4, tolerance 2e-2).
  - A-side one-hot: 4 rows per tensor_tensor broadcast instr (2x DVE mode).
  - C-side: gpsimd local_scatter builds a 2-row mask pair in one Pool op
    (dst[p, 792*(r%2) + lo*8 + c] = v; zeroing is internal; indices are
    collision-free since each chunk owns slot c mod 8); remaining rows use
    fused per-chunk tensor_scalar on DVE (4x mode) or 2-op ACT abs/relu.
  - fp16 hist staging + fp16 DRAM output (host converts to f32).
  - emission-level software pipelining: tile t+1's DMA/transpose/wide-math
    thunks are interleaved between tile t's row groups so Pool/PE never
    starve at tile boundaries.
"""

import sys

sys.path.insert(0, "/opt/trn_rl_repo")

from collections import deque

import numpy as np

import concourse.bass as bass
import concourse.tile as tile
from concourse import bacc, library_config, mybir
from concourse.bass_utils import run_bass_kernel_spmd
from concourse.masks import make_identity

N_CORES = 8
B, P = 4096, 1024
NUM_BINS = 9900
H, L = 100, 99
RT = 128  # rows per row-tile
NCHUNK = P // 128

f32 = mybir.dt.float32
f16 = mybir.dt.float16
i32 = mybir.dt.int32
i16 = mybir.dt.int16

INV99 = float(np.float32(1.0) / np.float32(99.0))
# u / 0.1f == u * 10 * (1 - EPS_D) exactly:  10*0.1f = 1 + 1.49e-8
EPS_D = float(1.0 - 1.0 / (10.0 * np.float64(np.float32(0.1))))

ALU = mybir.AluOpType
ACTF = mybir.ActivationFunctionType


def _in_pat(idx, pat):
    den, members = pat
    return (idx % den) in members


def build_program(
    rows_per_core: int,
    c_scat_pat: tuple = (8, (0, 2, 3, 5, 6)),  # row-pairs via Pool local_scatter
    c_act_pat: tuple = (5, (4,)),      # rows whose C' goes to ACT (2-op)
    a_batch: int = 4,                  # rows per A-eq instruction
    stage_rows: int = 2,
    cc_bufs: int = 32,
    a_bufs: int = 4,
    usub_act: bool = True,             # uT = pz-10 on ACT
    hi16_act: bool = True,             # hi16 convert on ACT
    stage_pat: tuple = (1, (0,)),      # stage groups on ACT (else DVE)
    mm_bufs: int = 3,
    scat_bufs: int = 3,
    wide_act_copies: bool = True,
):
    from contextlib import ExitStack

    assert rows_per_core % RT == 0
    nt = rows_per_core // RT
    assert a_batch in (1, 2, 4) and RT % a_batch == 0

    nc = bacc.Bacc(
        "TRN2", target_bir_lowering=False, debug=False, num_devices=N_CORES
    )
    mz_d = nc.dram_tensor("mz", [rows_per_core, P], f32, kind="ExternalInput").ap()
    it_d = nc.dram_tensor(
        "intensities", [rows_per_core, P], f32, kind="ExternalInput"
    ).ap()
    out_d = nc.dram_tensor(
        "out", [rows_per_core, NUM_BINS], f16, kind="ExternalOutput"
    ).ap()

    AB = a_batch
    AW = AB * NCHUNK
    SCW = 2 * L * NCHUNK  # scatter dst width for a row pair (1584)

    with tile.TileContext(nc) as tc, ExitStack() as ctx:
        cpool = ctx.enter_context(tc.tile_pool(name="consts", bufs=1))
        inpool = ctx.enter_context(tc.tile_pool(name="inp", bufs=2))
        tpsum = ctx.enter_context(tc.tile_pool(name="tpsum", bufs=2, space="PSUM"))
        scr = ctx.enter_context(tc.tile_pool(name="scratch", bufs=1))
        live = ctx.enter_context(tc.tile_pool(name="live", bufs=2))
        maskp = ctx.enter_context(tc.tile_pool(name="amask", bufs=a_bufs))
        cmaskp = ctx.enter_context(tc.tile_pool(name="cmask", bufs=cc_bufs))
        mmpsum = ctx.enter_context(
            tc.tile_pool(name="mmpsum", bufs=mm_bufs, space="PSUM")
        )
        histp = ctx.enter_context(tc.tile_pool(name="hist", bufs=2))
        scatp = ctx.enter_context(tc.tile_pool(name="cscat", bufs=scat_bufs))

        # ---- constants (standard gpsimd library ops all happen here) ----
        ident = cpool.tile([128, 128], f32, tag="ident")
        make_identity(nc, ident[:])
        iota_i = cpool.tile([128, H], i32, tag="iota_i")
        nc.gpsimd.iota(iota_i[:], pattern=[[1, H]], base=0, channel_multiplier=0)
        iota16 = cpool.tile([128, H], f16, tag="iota16")
        nc.vector.tensor_copy(iota16[:], iota_i[:])
        neg10 = cpool.tile([128, 1], f32, tag="neg10")
        nc.vector.memset(neg10[:], -10.0)
        # iotaA: value j at free position j*AW + k (batched A-eq comparand)
        iotaA = cpool.tile([128, H * AW], f16, tag="iotaA")
        nc.vector.tensor_copy(
            iotaA[:].rearrange("p (j k) -> p j k", k=AW),
            iota16[:].rearrange("p (j one) -> p j one", one=1).to_broadcast(
                [128, H, AW]
            ),
        )
        # scatter index offsets: value c + 792*(r%2) at free pos r*8+c
        cpat_i = cpool.tile([128, P], i32, tag="cpat_i")
        nc.gpsimd.iota(
            cpat_i[:], pattern=[[0, RT // 2], [L * NCHUNK, 2], [1, NCHUNK]],
            base=0, channel_multiplier=0,
        )
        cpat_f = cpool.tile([128, P], f32, tag="cpat_f")
        nc.vector.tensor_copy(cpat_f[:], cpat_i[:])
        # switch Pool ucode to local_scatter for the rest of the program
        nc.gpsimd.load_library(library_config.local_scatter)

        out_v = out_d.rearrange("(t r) (h l) -> t h r l", r=RT, l=L)

        def make_prep_thunks(t, st):
            """Emit-thunks for tile t's input DMA, transposes and wide math.
            Results land in st ('loT','vT','hi16','idx16','v16','nvT')."""
            th = []

            def t_dma():
                st["mzt"] = inpool.tile([128, P], f32, tag="mz")
                nc.sync.dma_start(st["mzt"][:], mz_d[t * RT : (t + 1) * RT, :])
                st["itt"] = inpool.tile([128, P], f32, tag="it")
                nc.sync.dma_start(st["itt"][:], it_d[t * RT : (t + 1) * RT, :])
                st["uT"] = scr.tile([128, P], f32, tag="uT")
                st["sqT"] = scr.tile([128, P], f32, tag="sqT")

            th.append(t_dma)

            def tr_mz(g):
                uT_v = st["uT"][:].rearrange("p (r c) -> p c r", c=NCHUNK)
                pz = tpsum.tile([128, 512], f32, tag="tp")
                for i, c in enumerate(range(g * 4, g * 4 + 4)):
                    nc.tensor.transpose(
                        pz[:, i * 128 : (i + 1) * 128],
                        st["mzt"][:, c * 128 : (c + 1) * 128],
                        ident[:],
                    )
                pz_v = pz[:].rearrange("p (q r) -> p q r", r=128)
                if usub_act:
                    nc.scalar.activation(
                        uT_v[:, g * 4 : g * 4 + 4, :], pz_v, ACTF.Identity,
                        bias=neg10[:], scale=1.0,
                    )
                else:
                    nc.vector.tensor_scalar(
                        uT_v[:, g * 4 : g * 4 + 4, :], pz_v, 10.0, None,
                        ALU.subtract,
                    )

            def tr_it(g):
                sqT_v = st["sqT"][:].rearrange("p (r c) -> p c r", c=NCHUNK)
                pz2 = tpsum.tile([128, 512], f32, tag="tp")
                for i, c in enumerate(range(g * 4, g * 4 + 4)):
                    nc.tensor.transpose(
                        pz2[:, i * 128 : (i + 1) * 128],
                        st["itt"][:, c * 128 : (c + 1) * 128],
                        ident[:],
                    )
                pz2_v = pz2[:].rearrange("p (q r) -> p q r", r=128)
                nc.scalar.sqrt(sqT_v[:, g * 4 : g * 4 + 4, :], pz2_v)

            for g in range(2):
                th.append(lambda g=g: tr_mz(g))
                th.append(lambda g=g: tr_it(g))

            def w(f):
                th.append(f)

            def w_b2():
                st["b2"] = scr.tile([128, P], f32, tag="b2")
                nc.vector.tensor_scalar(
                    st["b2"][:], st["uT"][:], 2.0, None, ALU.mult
                )

            def w_qhi():
                st["qhi"] = scr.tile([128, P], f32, tag="qhi")
                nc.vector.tensor_scalar(
                    st["qhi"][:], st["uT"][:], 10.0, None, ALU.mult
                )

            def w_bv():
                st["bv"] = scr.tile([128, P], f32, tag="bv")
                nc.vector.scalar_tensor_tensor(
                    st["bv"][:], in0=st["uT"][:], scalar=-8.0,
                    in1=st["qhi"][:], op0=ALU.mult, op1=ALU.add,
                )

            def w_ederr():
                st["ederr"] = scr.tile([128, P], f32, tag="ederr")
                nc.vector.tensor_tensor(
                    st["ederr"][:], st["b2"][:], st["bv"][:], ALU.subtract
                )

            def w_c2t():
                st["c2t"] = scr.tile([128, P], f32, tag="bv")
                nc.vector.scalar_tensor_tensor(
                    st["c2t"][:], in0=st["qhi"][:], scalar=-EPS_D,
                    in1=st["ederr"][:], op0=ALU.mult, op1=ALU.add,
                )

            def w_qT():
                st["qT"] = scr.tile([128, P], f32, tag="qT")
                nc.vector.tensor_tensor(
                    st["qT"][:], st["qhi"][:], st["c2t"][:], ALU.add
                )

            def w_itmp():
                st["itmp"] = scr.tile([128, P], i32, tag="itmp")
                nc.vector.tensor_copy(st["itmp"][:], st["qT"][:])

            def w_ftmp():
                st["ftmp"] = scr.tile([128, P], f32, tag="ftmp")
                (nc.scalar.copy if wide_act_copies else nc.vector.tensor_copy)(
                    st["ftmp"][:], st["itmp"][:]
                )

            def w_cond():
                st["cond"] = scr.tile([128, P], f32, tag="b2")
                nc.vector.tensor_tensor(
                    st["cond"][:], st["ftmp"][:], st["qT"][:], ALU.is_gt
                )

            def w_binT():
                st["binT"] = scr.tile([128, P], f32, tag="qhi")
                nc.vector.tensor_tensor(
                    st["binT"][:], st["ftmp"][:], st["cond"][:], ALU.subtract
                )

            def w_hq():
                st["hq"] = scr.tile([128, P], f32, tag="qT")
                nc.vector.tensor_scalar(
                    st["hq"][:], st["binT"][:], 0.5, INV99, ALU.add, ALU.mult
                )

            def w_itmp2():
                st["itmp2"] = scr.tile([128, P], i32, tag="itmp")
                nc.vector.tensor_copy(st["itmp2"][:], st["hq"][:])

            def w_ftmp2():
                st["ftmp2"] = scr.tile([128, P], f32, tag="ftmp")
                (nc.scalar.copy if wide_act_copies else nc.vector.tensor_copy)(
                    st["ftmp2"][:], st["itmp2"][:]
                )

            def w_cond2():
                st["cond2"] = scr.tile([128, P], f32, tag="b2")
                nc.vector.tensor_tensor(
                    st["cond2"][:], st["ftmp2"][:], st["hq"][:], ALU.is_gt
                )

            def w_hiT():
                st["hiT"] = scr.tile([128, P], f32, tag="ederr")
                nc.vector.tensor_tensor(
                    st["hiT"][:], st["ftmp2"][:], st["cond2"][:], ALU.subtract
                )

            def w_loT():
                st["loT"] = live.tile([128, P], f32, tag="loT")
                nc.vector.scalar_tensor_tensor(
                    st["loT"][:], in0=st["hiT"][:], scalar=-99.0,
                    in1=st["binT"][:], op0=ALU.mult, op1=ALU.add,
                )

            def w_v1():
                st["v1"] = scr.tile([128, P], f32, tag="v1")
                nc.vector.scalar_tensor_tensor(
                    st["v1"][:], in0=st["uT"][:], scalar=0.0,
                    in1=st["sqT"][:], op0=ALU.is_ge, op1=ALU.mult,
                )

            def w_vT():
                st["vT"] = live.tile([128, P], f32, tag="vT")
                nc.vector.scalar_tensor_tensor(
                    st["vT"][:], in0=st["uT"][:], scalar=990.0,
                    in1=st["v1"][:], op0=ALU.is_lt, op1=ALU.mult,
                )

            def w_hi16():
                st["hi16"] = live.tile([128, P], f16, tag="hi16")
                (nc.scalar.copy if hi16_act else nc.vector.tensor_copy)(
                    st["hi16"][:], st["hiT"][:]
                )

            def w_idxf():
                st["idxf"] = scr.tile([128, P], f32, tag="uT")
                nc.vector.scalar_tensor_tensor(
                    st["idxf"][:], in0=st["loT"][:], scalar=8.0,
                    in1=cpat_f[:], op0=ALU.mult, op1=ALU.add,
                )

            def w_idx16():
                st["idx16"] = live.tile([128, P], i16, tag="idx16")
                nc.vector.tensor_copy(st["idx16"][:], st["idxf"][:])

            def w_v16():
                st["v16"] = live.tile([128, P], f16, tag="v16")
                nc.vector.tensor_copy(st["v16"][:], st["vT"][:])

            def w_nvT():
                st["nvT"] = live.tile([128, P], f32, tag="nvT")
                nc.vector.tensor_scalar(
                    st["nvT"][:], st["vT"][:], -1.0, None, ALU.mult
                )

            for f in (w_b2, w_qhi, w_bv, w_ederr, w_c2t, w_qT, w_itmp,
                      w_ftmp, w_cond, w_binT, w_hq, w_itmp2, w_ftmp2,
                      w_cond2, w_hiT, w_loT, w_v1, w_vT, w_hi16, w_idxf,
                      w_idx16, w_v16):
                w(f)
            if c_act_pat[1]:
                w(w_nvT)
            return th

        states = [dict() for _ in range(nt)]
        for f in make_prep_thunks(0, states[0]):
            f()

        ngroups = RT // AB
        for t in range(nt):
            st = states[t]
            nxt = (
                deque(make_prep_thunks(t + 1, states[t + 1]))
                if t + 1 < nt
                else deque()
            )
            nxt_total = len(nxt)
            nxt_done = 0

            hist = histp.tile([H, RT * H], f16, tag="hist")
            hist_v = hist[:].rearrange("h (r l) -> h r l", l=H)
            loT, vT = st["loT"], st["vT"]
            hi16, idx16, v16 = st["hi16"], st["idx16"], st["v16"]
            pr_full = None
            for gidx in range(ngroups):
                r0 = gidx * AB
                a4 = maskp.tile([128, H * AW], f16, tag="A")
                nc.vector.tensor_tensor(
                    a4[:].rearrange("p (j k) -> p j k", k=AW),
                    iotaA[:].rearrange("p (j k) -> p j k", k=AW),
                    hi16[:, r0 * NCHUNK : r0 * NCHUNK + AW]
                    .rearrange("p (one k) -> p one k", one=1)
                    .to_broadcast([128, H, AW]),
                    ALU.is_equal,
                )
                a4_v = a4[:].rearrange("p (j k) -> p j k", k=AW)
                for r in range(r0, r0 + AB):
                    if r % stage_rows == 0:
                        pr_full = mmpsum.tile(
                            [H, stage_rows * 512], f32, tag="mm"
                        )
                    pr = pr_full[:, (r % stage_rows) * 512 :][:, :L]
                    pair = r // 2
                    use_scat = _in_pat(pair, c_scat_pat)
                    if use_scat and r % 2 == 0:
                        sc = scatp.tile([128, SCW], f16, tag="S")
                        nc.gpsimd.local_scatter(
                            sc[:],
                            v16[:, r * NCHUNK : (r + 2) * NCHUNK],
                            idx16[:, r * NCHUNK : (r + 2) * NCHUNK],
                            channels=128,
                            num_elems=SCW,
                            num_idxs=2 * NCHUNK,
                        )
                        st["sc_v"] = sc[:].rearrange(
                            "p (b j c) -> p b j c", b=2, c=NCHUNK
                        )
                    for c in range(NCHUNK):
                        col = r * NCHUNK + c
                        if use_scat:
                            rhs = st["sc_v"][:, r % 2, :, c]
                        elif _in_pat(r, c_act_pat):
                            cc = cmaskp.tile([128, H], f16, tag="C")
                            at = cmaskp.tile([128, H], f16, tag="Ct")
                            nc.scalar.activation(
                                at[:], iota16[:], ACTF.Abs,
                                bias=loT[:, col : col + 1], scale=-1.0,
                            )
                            nc.scalar.activation(
                                cc[:], at[:], ACTF.Relu,
                                bias=vT[:, col : col + 1],
                                scale=st["nvT"][:, col : col + 1],
                            )
                            rhs = cc[:, 0:L]
                        else:
                            cc = cmaskp.tile([128, H], f16, tag="C")
                            nc.vector.tensor_scalar(
                                cc[:], iota16[:],
                                loT[:, col : col + 1], vT[:, col : col + 1],
                                ALU.is_equal, ALU.mult,
                            )
                            rhs = cc[:, 0:L]
                        nc.tensor.matmul(
                            pr[:, :],
                            lhsT=a4_v[:, :, (r - r0) * NCHUNK + c],
                            rhs=rhs,
                            start=(c == 0),
                            stop=(c == NCHUNK - 1),
                        )
                    if (r + 1) % stage_rows == 0:
                        rs = r + 1 - stage_rows
                        src = pr_full[:].rearrange(
                            "h (b x) -> h b x", x=512
                        )[:, :, 0:L]
                        if _in_pat(r // stage_rows, stage_pat):
                            nc.scalar.copy(hist_v[:, rs : r + 1, 0:L], src)
                        else:
                            nc.vector.tensor_copy(
                                hist_v[:, rs : r + 1, 0:L], src
                            )
                # interleave next tile's prep thunks across the groups
                want = (nxt_total * (gidx + 1)) // ngroups
                while nxt_done < want:
                    nxt.popleft()()
                    nxt_done += 1

            nc.sync.dma_start(out_v[t], hist_v[:, :, 0:L])

    nc.compile()
    return nc


_CACHE: dict = {}


def _get_program(rows_per_core: int, **cfg):
    key = (rows_per_core, tuple(sorted(cfg.items())))
    if key not in _CACHE:
        _CACHE[key] = build_program(rows_per_core, **cfg)
    return _CACHE[key]


def kernel(mz: np.ndarray, intensities: np.ndarray, trace: bool = False):
    mz = np.ascontiguousarray(np.asarray(mz, dtype=np.float32))
    intensities = np.ascontiguousarray(np.asarray(intensities, dtype=np.float32))
    bb = mz.shape[0]
    rows_per_core = bb // N_CORES
    nc = _get_program(rows_per_core)
    in_maps = []
    for i in range(N_CORES):
        sl = slice(i * rows_per_core, (i + 1) * rows_per_core)
        in_maps.append({"mz": mz[sl], "intensities": intensities[sl]})
    try:
        res = run_bass_kernel_spmd(
            nc, in_maps, core_ids=list(range(N_CORES)), trace=trace
        )
    except ModuleNotFoundError:
        res = run_bass_kernel_spmd(
            nc, in_maps, core_ids=list(range(N_CORES)), trace=False
        )
    out = np.concatenate(
        [res.results[i]["out"] for i in range(N_CORES)], axis=0
    ).astype(np.float32)
    if trace:
        kernel.last_exec_time_ns = res.exec_time_ns
        kernel.last_results = res
    return out


kernel.last_exec_time_ns = None
